# revision 14
# baseline (speedup 1.0000x reference)
"""Trainium2 Bass kernel for MultiHeadedAttention (B=2, S=2048, DM=1024, H=16).

Sharding: batch x heads across 8 cores. Core c handles batch c//4, heads
(c%4)*4 .. +4. Each core returns its 4 heads' p_attn and a partial output
projection; host sums partials per batch and assembles p_attn.

Returns (out [2,2048,1024] f32, p_attn [2,16,2048,2048] f32) matching the
reference tuple.
"""
import sys
import numpy as np

for _p in ("/opt/trn_rl_repo",):
    if _p not in sys.path:
        sys.path.append(_p)

B, S, DM, H, DK = 2, 2048, 1024, 16, 64
HL = 4            # heads per core
NC = 8            # cores
NK = DM // 128    # 8 contraction chunks for projections
NI = S // 128     # 16 i-chunks
CW = HL * DK      # 256 local projection width
SCALE = 1.0 / 8.0  # 1/sqrt(DK)

_CACHE = {}
DEBUG = False


def _build():
    import concourse.bass as bass
    import concourse.bacc as bacc
    import concourse.mybir as mybir
    import concourse.tile as tile

    FP32 = mybir.dt.float32
    FP32R = mybir.dt.float32r
    AF = mybir.ActivationFunctionType
    ALU = mybir.AluOpType

    nc = bacc.Bacc(None, target_bir_lowering=False, debug=False)
    with tile.TileContext(nc) as tc:
        with tc.tile_pool(name="dram", bufs=1, space="DRAM") as dram:
            xqT = dram.tile([DM, S], FP32, kind="ExternalInput", uniquify=False, name="xqT")
            xkT = dram.tile([DM, S], FP32, kind="ExternalInput", uniquify=False, name="xkT")
            xvT = dram.tile([DM, S], FP32, kind="ExternalInput", uniquify=False, name="xvT")
            wq = dram.tile([DM, CW], FP32, kind="ExternalInput", uniquify=False, name="wq")
            wk = dram.tile([DM, CW], FP32, kind="ExternalInput", uniquify=False, name="wk")
            wv = dram.tile([DM, CW], FP32, kind="ExternalInput", uniquify=False, name="wv")
            wo = dram.tile([CW, DM], FP32, kind="ExternalInput", uniquify=False, name="wo")
            bq = dram.tile([1, CW], FP32, kind="ExternalInput", uniquify=False, name="bq")
            bk = dram.tile([1, CW], FP32, kind="ExternalInput", uniquify=False, name="bk")
            bv = dram.tile([1, CW], FP32, kind="ExternalInput", uniquify=False, name="bv")
            p_out = dram.tile([HL, S, S], FP32R, kind="ExternalOutput", uniquify=False, name="p_out")
            zvec = dram.tile([HL, S], FP32, kind="ExternalOutput", uniquify=False, name="zvec")
            o_part = dram.tile([S, DM], FP32, kind="ExternalOutput", uniquify=False, name="o_part")
            if DEBUG:
                dbg_v = dram.tile([16, 128, HL * 65], FP32R, kind="ExternalOutput", uniquify=False, name="dbg_v")
                dbg_xhT = dram.tile([128, 2 * S], FP32R, kind="ExternalOutput", uniquify=False, name="dbg_xhT")
                dbg_z = dram.tile([HL, S], FP32, kind="ExternalOutput", uniquify=False, name="dbg_z")

            with tc.tile_pool(name="consts", bufs=1) as consts, \
                 tc.tile_pool(name="wpool", bufs=2) as wpool, \
                 tc.tile_pool(name="stag", bufs=2) as stag, \
                 tc.tile_pool(name="xrp", bufs=3) as xrp, \
                 tc.tile_pool(name="qkv", bufs=1) as qkv, \
                 tc.tile_pool(name="putp", bufs=2) as putp, \
                 tc.tile_pool(name="pup", bufs=2) as pup, \
                 tc.tile_pool(name="zp", bufs=2) as zp, \
                 tc.tile_pool(name="xsc", bufs=1) as xsc, \
                 tc.tile_pool(name="outp", bufs=2) as outp, \
                 tc.tile_pool(name="ps_sm", bufs=4, space="PSUM") as ps_sm, \
                 tc.tile_pool(name="ps_lg", bufs=2, space="PSUM") as ps_lg:

                # ---------------- constants ----------------
                ones_f = stag.tile([1, S], FP32, name="ones_f", tag="xstag")
                nc.vector.memset(ones_f[:], 1.0)
                ones_row = consts.tile([1, S], FP32R, name="ones_row")
                nc.vector.tensor_copy(ones_row[:], ones_f[:])
                ones_c4 = consts.tile([128, HL], FP32, name="ones_c4")
                nc.vector.memset(ones_c4[:], 1.0)
                zero_f = stag.tile([1, S], FP32, name="zero_f", tag="xstag")
                nc.vector.memset(zero_f[:], 0.0)
                zero_row = consts.tile([1, S], FP32R, name="zero_row")
                nc.vector.tensor_copy(zero_row[:], zero_f[:])

                # ---------------- load + cast weights ----------------
                # wq/wk/wv: [DM, CW] -> 8 chunks [128, CW] stored along free dim
                def load_w(nm, dt_, ncols, nchunks):
                    wtile = wpool.tile([128, NK * CW], FP32R, name=f"{nm}r", tag="w")
                    for kc in range(nchunks):
                        nc.gpsimd.dma_start(wtile[:, kc * ncols:(kc + 1) * ncols],
                                            dt_[kc * 128:(kc + 1) * 128, :])
                    return wtile
                br = {}
                for nm, dt_ in (("bq", bq), ("bk", bk), ("bv", bv)):
                    brow = consts.tile([1, CW], FP32R, name=f"{nm}r")
                    nc.gpsimd.dma_start(brow[:], dt_[:, :])
                    br[nm] = brow

                # ---------------- S1: projections ----------------
                # qT/kT: [CW=256, S] as [128, 2*S] (m-chunk m at cols m*S..)
                # psum wave: M0 -> 2 lg tiles [128,1024]; M1 -> 4 sm tiles [128,512]
                qkT = {}
                for nm, xsrc, wdram in (("q", xqT, wq), ("k", xkT, wk)):
                    lg0 = ps_lg.tile([128, 1024], FP32, name=f"lg0_{nm}", tag="lg")
                    lg1 = ps_lg.tile([128, 1024], FP32, name=f"lg1_{nm}", tag="lg")
                    sm = [ps_sm.tile([128, 512], FP32, name=f"sm{n}_{nm}", tag="sm")
                          for n in range(4)]
                    wt = load_w("w" + nm, wdram, CW, NK)
                    for kc in range(NK):
                        xr = xrp.tile([128, S], FP32R, name=f"xr_{nm}{kc}", tag="xr")
                        nc.gpsimd.dma_start(xr[:], xsrc[kc * 128:(kc + 1) * 128, :])
                        for n in range(4):
                            # M0 -> lg halves, M1 -> sm
                            dst0 = (lg0 if n < 2 else lg1)[:, (n % 2) * 512:(n % 2) * 512 + 512]
                            nc.tensor.matmul(dst0, wt[:, kc * CW:kc * CW + 128],
                                             xr[:, n * 512:(n + 1) * 512],
                                             start=(kc == 0), stop=False)
                            nc.tensor.matmul(sm[n][:], wt[:, kc * CW + 128:kc * CW + 256],
                                             xr[:, n * 512:(n + 1) * 512],
                                             start=(kc == 0), stop=False)
                    # bias rows (K=1), also closes accumulation groups
                    brow = br["b" + nm]
                    for n in range(4):
                        dst0 = (lg0 if n < 2 else lg1)[:, (n % 2) * 512:(n % 2) * 512 + 512]
                        nc.tensor.matmul(dst0, brow[:, 0:128],
                                         ones_row[:, n * 512:(n + 1) * 512],
                                         start=False, stop=True)
                        nc.tensor.matmul(sm[n][:], brow[:, 128:256],
                                         ones_row[:, n * 512:(n + 1) * 512],
                                         start=False, stop=True)
                    dst = qkv.tile([128, 2 * S], FP32R, name=f"{nm}Tr", tag=f"{nm}Tr")
                    nc.vector.tensor_copy(dst[:, 0:1024], lg0[:])
                    nc.vector.tensor_copy(dst[:, 1024:2048], lg1[:])
                    for n in range(4):
                        nc.vector.tensor_copy(dst[:, S + n * 512:S + (n + 1) * 512], sm[n][:])
                    qkT[nm] = dst

                # v: [S, CW] natural; 16 s-chunks [128, 256] -> vr tiles [128, 260]
                # (per head: 64 v cols + ones col for the Z row)
                # 16 [128,256] accumulators packed 2-per-sm-slot, 4-per-lg-slot
                vps = []
                sm_v = [ps_sm.tile([128, 512], FP32, name=f"vsm{i}", tag="sm") for i in range(4)]
                lg_v = [ps_lg.tile([128, 1024], FP32, name=f"vlg{i}", tag="lg") for i in range(2)]
                for sc in range(16):
                    if sc < 8:
                        t = sm_v[sc // 2]
                        vps.append(t[:, (sc % 2) * 256:(sc % 2) * 256 + 256])
                    else:
                        s2 = sc - 8
                        t = lg_v[s2 // 4]
                        vps.append(t[:, (s2 % 4) * 256:(s2 % 4) * 256 + 256])
                wvr = load_w("wv", wv, CW, NK)
                for t in sm_v:
                    nc.tensor.matmul(t[:, 0:512], zero_row[:, 0:128], zero_row[:, 0:512],
                                     start=True, stop=False)
                for t in lg_v:
                    for half in range(2):
                        nc.tensor.matmul(t[:, half * 512:(half + 1) * 512],
                                         zero_row[:, 0:128], zero_row[:, 0:512],
                                         start=True, stop=False)
                for kc in range(NK):
                    st = stag.tile([128, S], FP32, name=f"x_v{kc}", tag="xstag")
                    nc.sync.dma_start(st[:], xvT[kc * 128:(kc + 1) * 128, :])
                    xr = xrp.tile([128, S], FP32R, name=f"xr_v{kc}", tag="xr")
                    nc.vector.tensor_copy(xr[:], st[:])
                    for sc in range(16):
                        nc.tensor.matmul(vps[sc], xr[:, sc * 128:(sc + 1) * 128],
                                         wvr[:, kc * CW:(kc + 1) * CW],
                                         start=False, stop=False)
                for sc in range(16):
                    nc.tensor.matmul(vps[sc], ones_row[:, sc * 128:sc * 128 + 128],
                                     br["bv"][:], start=False, stop=True)
                vr = []
                for sc in range(16):
                    vt = qkv.tile([128, HL * 65], FP32R, name=f"vr{sc}", tag=f"vr{sc}")
                    src3 = vps[sc].rearrange("p (h c) -> p h c", c=64)
                    dst3 = vt.rearrange("p (h c) -> p h c", c=65)
                    nc.vector.tensor_copy(dst3[:, :, 0:64], src3[:, :, :])
                    nc.vector.tensor_copy(dst3[:, :, 64:65],
                                          ones_c4.rearrange("p (h o) -> p h o", o=1)[:, :, :])
                    vr.append(vt)
                    if DEBUG:
                        nc.sync.dma_start(dbg_v[sc, :, :], vt[:])

                # ---------------- S2 + S3 per head ----------------
                qT, kT = qkT["q"], qkT["k"]
                for h in range(HL):
                    po = (h % 2) * 64        # partition offset within m-chunk
                    co = (h // 2) * S        # free-col offset of m-chunk
                    # ---- S3: transposed scores -> exp -> AV (+Z row via ones col)
                    xps = [ps_sm.tile([128, 512], FP32, name=f"x{h}_{it}", tag="sm")
                           for it in range(4)]
                    for jc in range(NI):
                        lhsj = kT[po:po + 64, co + jc * 128: co + jc * 128 + 128]
                        puT = putp.tile([128, S], FP32R, name=f"puT{h}_{jc}", tag="puT")
                        for half in range(2):
                            sT = ps_lg.tile([128, 1024], FP32, name=f"sT{h}_{jc}_{half}", tag="lg")
                            for n in range(2):
                                i0 = half * 1024 + n * 512
                                nc.tensor.matmul(sT[:, n * 512:(n + 1) * 512], lhsj,
                                                 qT[po:po + 64, co + i0: co + i0 + 512],
                                                 start=True, stop=True)
                            nc.scalar.activation(puT[:, half * 1024:(half + 1) * 1024], sT[:],
                                                 AF.Exp, scale=SCALE)
                        nc.sync.dma_start(p_out[h, jc * 128:(jc + 1) * 128, :], puT[:])
                        for it in range(4):
                            nc.tensor.matmul(xps[it][0:65, :],
                                             vr[jc][:, h * 65:(h + 1) * 65],
                                             puT[:, it * 512:(it + 1) * 512],
                                             start=(jc == 0), stop=(jc == NI - 1))
                    # ---- normalize x by Z (row 64 of each xps) and store transposed
                    xu = xsc.tile([65, S], FP32, name=f"xu{h}", tag="xu")
                    for it in range(4):
                        nc.vector.tensor_copy(xu[:, it * 512:(it + 1) * 512], xps[it][0:65, :])
                    zrow = xu[64:65, :]
                    nc.sync.dma_start(zvec[h:h + 1, :], zrow)
                    zrec = xsc.tile([1, S], FP32, name=f"zrec{h}", tag="zrec")
                    nc.vector.reciprocal(zrec[:], zrow)
                    zb = xsc.tile([64, S], FP32, name=f"zb{h}", tag="zb")
                    nc.gpsimd.partition_broadcast(zb[:], zrec[:])
                    if DEBUG:
                        nc.sync.dma_start(dbg_z[h:h + 1, :], zrec[:])
                    xh = xsc.tile([64, S], FP32, name=f"xh{h}", tag="xh")
                    nc.vector.tensor_tensor(xh[:], xu[0:64, :], zb[:], op=ALU.mult)
                    if h == 0:
                        xheadsT = qkv.tile([128, 2 * S], FP32R, name="xheadsT", tag="xheadsT")
                    nc.vector.tensor_copy(
                        xheadsT[(h % 2) * 64:(h % 2) * 64 + 64, (h // 2) * S:(h // 2) * S + S],
                        xh[:])

                if DEBUG:
                    nc.sync.dma_start(dbg_xhT[:, :], xheadsT[:])
                # ---------------- S4: output projection (partial) ----------------
                wor = load_w("wo", wo, DM, 2)
                for ic in range(NI):
                    ops = ps_lg.tile([128, 1024], FP32, name=f"o{ic}", tag="lg")
                    for kc2 in range(2):
                        for n in range(2):
                            nc.tensor.matmul(ops[:, n * 512:(n + 1) * 512],
                                             xheadsT[:, kc2 * S + ic * 128: kc2 * S + ic * 128 + 128],
                                             wor[:, kc2 * DM + n * 512: kc2 * DM + (n + 1) * 512],
                                             start=(kc2 == 0), stop=(kc2 == 1))
                    osb = outp.tile([128, DM], FP32, name=f"osb{ic}", tag="osb")
                    nc.vector.tensor_copy(osb[:], ops[:])
                    nc.sync.dma_start(o_part[ic * 128:(ic + 1) * 128, :], osb[:])
    nc.compile()
    return nc


def _numpy_fallback(query, key, value, mask, wq, bq, wk, bk, wv, bv, wo, bo):
    def proj(x, w, b):
        return (x @ w + b).reshape(B, S, H, DK).transpose(0, 2, 1, 3)
    q = proj(query, wq, bq)
    k = proj(key, wk, bk)
    v = proj(value, wv, bv)
    scores = np.einsum("bhqd,bhkd->bhqk", q, k) / np.sqrt(DK).astype(np.float32)
    scores = np.where(mask == 0, -np.inf, scores)
    m = scores.max(-1, keepdims=True)
    e = np.exp(scores - m)
    p = e / e.sum(-1, keepdims=True)
    x = np.einsum("bhqk,bhkd->bhqd", p, v)
    x = x.transpose(0, 2, 1, 3).reshape(B, S, H * DK)
    return (x @ wo + bo).astype(np.float32), p.astype(np.float32)


def kernel(query, key, value, mask, wq, bq, wk, bk, wv, bv, wo, bo, _results_hook=None, _trace=False, _trace_cores=None):
    query = np.asarray(query, dtype=np.float32)
    key = np.asarray(key, dtype=np.float32)
    value = np.asarray(value, dtype=np.float32)
    mask = np.asarray(mask)
    wq, wk, wv, wo = (np.asarray(a, dtype=np.float32) for a in (wq, wk, wv, wo))
    bq, bk, bv, bo = (np.asarray(a, dtype=np.float32) for a in (bq, bk, bv, bo))

    if not np.all(mask != 0):
        return _numpy_fallback(query, key, value, mask, wq, bq, wk, bk, wv, bv, wo, bo)

    if "nc" not in _CACHE:
        _CACHE["nc"] = _build()
    nc = _CACHE["nc"]

    in_maps = []
    for c in range(NC):
        b = c // 4
        h0 = (c % 4) * HL
        cs = slice(h0 * DK, (h0 + HL) * DK)
        in_maps.append({
            "xqT": np.ascontiguousarray(query[b].T),
            "xkT": np.ascontiguousarray(key[b].T),
            "xvT": np.ascontiguousarray(value[b].T),
            "wq": np.ascontiguousarray(wq[:, cs]),
            "wk": np.ascontiguousarray(wk[:, cs]),
            "wv": np.ascontiguousarray(wv[:, cs]),
            "wo": np.ascontiguousarray(wo[cs, :]),
            "bq": np.ascontiguousarray(bq[cs]).reshape(1, CW),
            "bk": np.ascontiguousarray(bk[cs]).reshape(1, CW),
            "bv": np.ascontiguousarray(bv[cs]).reshape(1, CW),
        })

    from concourse.bass_utils import run_bass_kernel_spmd
    kw = {}
    if _trace:
        kw = dict(trace=True, trace_cores=_trace_cores or [0])
    res = run_bass_kernel_spmd(nc, in_maps, core_ids=list(range(NC)), **kw)
    if _results_hook is not None:
        _results_hook(res)
    r = res.results

    p_attn = np.empty((B, H, S, S), dtype=np.float32)
    for c in range(NC):
        b = c // 4
        h0 = (c % 4) * HL
        puT = r[c]["p_out"]          # [HL, S(j), S(i)] unnormalized exp
        z = r[c]["zvec"]             # [HL, S(i)]
        for hl in range(HL):
            np.divide(puT[hl].T, z[hl][:, None], out=p_attn[b, h0 + hl])
    out = np.zeros((B, S, DM), dtype=np.float32)
    for c in range(NC):
        out[c // 4] += r[c]["o_part"]
    out += bo
    return out, p_attn
